# revision 8
# baseline (speedup 1.0000x reference)
"""Trainium2 Bass kernel for a ClassificationHead:
  h = x[:, 1:, :]                      # drop CLS token
  h = LayerNorm(h) * gamma + beta      # over last dim (768)
  logits = h @ W.T + bias              # W: [1, 768]
  out = sigmoid(logits)                # [256, 256, 1]

Math reformulation (lets everything run as per-token reductions):
  geff = gamma * W[0]
  G    = sum(geff)
  g2   = geff - G/768            # folds the mean-correction into the weights
  c    = dot(beta, W[0]) + bias[0]
  s2[t]   = dot(h[t], g2)
  var[t]  = population variance of h[t]   (bn_stats/bn_aggr)
  out[t]  = sigmoid(s2[t] / sqrt(var[t] + eps) + c)

Sharding: data-parallel over 8 NeuronCores, 32 batches (8192 tokens) per core.
Token tiling inside a core: tile column s holds tokens {64*p + s : p in 0..127}
so the final [128, 64] result tile stores contiguously to DRAM.
"""

import os

import numpy as np

import concourse.bacc as bacc
import concourse.bass as bass
import concourse.tile as tile
from concourse import mybir
from concourse.bass_utils import run_bass_kernel_spmd

B, N, E = 256, 257, 768
N_CORES = 8
BS = B // N_CORES          # batches per core
T = BS * (N - 1)           # tokens per core = 8192
P = 128                    # partitions
S = T // P                 # token-tile columns per core = 64
EPS = 1e-5

_CACHE = {}
LAST_RESULTS = None        # test harness reads exec_time_ns off this


def _build_nc(n_cols=S):
    S_ = n_cols
    T_ = P * S_
    nc = bacc.Bacc(None, target_bir_lowering=False)
    f32 = mybir.dt.float32

    x = nc.dram_tensor("x", [T_, E], f32, kind="ExternalInput")
    # params: [:, :768] = g2 replicated across partitions, [:, 768] = c
    params = nc.dram_tensor("params", [P, E + 1], f32, kind="ExternalInput")
    out = nc.dram_tensor("out", [T_], f32, kind="ExternalOutput")

    # x_r[s, p, e] = x[64*p + s, e]
    x_r = x.ap().rearrange("(p s) e -> s p e", p=P)
    out_r = out.ap().rearrange("(p s) -> p s", p=P)

    with tile.TileContext(nc) as tc:
        with (
            tc.tile_pool(name="singles", bufs=1) as singles,
            tc.tile_pool(name="loads", bufs=4) as loads,
            tc.tile_pool(name="work", bufs=4) as work,
            tc.tile_pool(name="stats", bufs=1) as stats_pool,
        ):
            params_t = singles.tile([P, E + 1], f32)
            nc.sync.dma_start(out=params_t, in_=params.ap())
            g2_t = params_t[:, 0:E]
            c_ap = params_t[:, E : E + 1]

            eps_t = singles.tile([P, 1], f32)
            nc.vector.memset(eps_t, EPS)

            mv = stats_pool.tile([P, S_, 2], f32)     # (mean, var) per token
            s2 = stats_pool.tile([P, S_], f32)        # dot(h, g2) per token

            n_sub = 3                                 # 768 = 3 * 256 (FMAX 512)
            fsub = E // n_sub

            for s in range(S_):
                x_t = loads.tile([P, E], f32)
                nc.sync.dma_start(out=x_t, in_=x_r[s])

                x3 = x_t.rearrange("p (g f) -> p g f", g=n_sub)
                st = work.tile([P, n_sub, 6], f32, tag="bnstats")
                for g in range(n_sub):
                    nc.vector.bn_stats(out=st[:, g, :], in_=x3[:, g, :])
                nc.vector.bn_aggr(out=mv[:, s, :], in_=st)

                dummy = work.tile([P, 1], f32, tag="prod")
                nc.vector.scalar_tensor_tensor(
                    out=dummy.broadcast_to(x_t.shape),
                    in0=x_t,
                    scalar=1.0,
                    in1=g2_t,
                    op0=mybir.AluOpType.mult,
                    op1=mybir.AluOpType.mult,
                    accum_out=s2[:, s : s + 1],
                )

            # epilogue, batched over all tokens of the core: [128, 64]
            std = stats_pool.tile([P, S_], f32)
            nc.scalar.activation(
                out=std,
                in_=mv[:, :, 1],
                func=mybir.ActivationFunctionType.Sqrt,
                bias=eps_t,
                scale=1.0,
            )
            r = stats_pool.tile([P, S_], f32)
            nc.vector.reciprocal(out=r, in_=std)
            logit = stats_pool.tile([P, S_], f32)
            nc.vector.tensor_mul(out=logit, in0=s2, in1=r)
            res = stats_pool.tile([P, S_], f32)
            nc.scalar.activation(
                out=res,
                in_=logit,
                func=mybir.ActivationFunctionType.Sigmoid,
                bias=c_ap,
                scale=1.0,
            )
            nc.sync.dma_start(out=out_r, in_=res)

    nc.compile()
    return nc


def kernel(x, ln_gamma, ln_beta, W, bias):
    global LAST_RESULTS
    x = np.ascontiguousarray(np.asarray(x, dtype=np.float32))
    ln_gamma = np.asarray(ln_gamma, dtype=np.float32)
    ln_beta = np.asarray(ln_beta, dtype=np.float32)
    W = np.asarray(W, dtype=np.float32)
    bias = np.asarray(bias, dtype=np.float32)

    geff = ln_gamma * W[0]
    g2 = geff - geff.sum() / E
    c = float(ln_beta @ W[0] + bias[0])

    params = np.empty((P, E + 1), dtype=np.float32)
    params[:, :E] = g2[None, :]
    params[:, E] = c

    # drop CLS, shard over cores, flatten to [T, E] per core
    h = x[:, 1:, :]                                  # [256, 256, 768]
    shards = [
        np.ascontiguousarray(h[i * BS : (i + 1) * BS].reshape(T, E))
        for i in range(N_CORES)
    ]

    if "nc" not in _CACHE:
        _CACHE["nc"] = _build_nc()
    nc = _CACHE["nc"]

    in_maps = [{"x": shards[i], "params": params} for i in range(N_CORES)]
    trace = bool(int(os.environ.get("BASS_KERNEL_TRACE", "0")))
    results = run_bass_kernel_spmd(
        nc, in_maps, core_ids=list(range(N_CORES)), trace=trace
    )
    LAST_RESULTS = results

    outs = [results.results[i]["out"] for i in range(N_CORES)]
    full = np.concatenate(outs).reshape(B, N - 1, 1).astype(np.float32)
    return full


# revision 10
# speedup vs baseline: 1.0307x; 1.0307x over previous
"""Trainium2 Bass kernel for a ClassificationHead:
  h = x[:, 1:, :]                      # drop CLS token
  h = LayerNorm(h) * gamma + beta      # over last dim (768)
  logits = h @ W.T + bias              # W: [1, 768]
  out = sigmoid(logits)                # [256, 256, 1]

Math reformulation (everything becomes per-token reductions over e=768):
  geff = gamma * W[0]
  g2   = geff - sum(geff)/768    # folds the LN mean-correction into the weights
  c    = dot(beta, W[0]) + bias[0]
  s2[t]  = dot(h[t], g2)         # DVE scalar_tensor_tensor accum
  sm[t]  = sum(h[t])             # DVE tensor_scalar accum / ACT Copy accum
  sq[t]  = sum(h[t]^2)           # ACT Square accum
  var[t] = sq/768 - (sm/768)^2
  out[t] = sigmoid(s2[t] / sqrt(var[t] + eps) + c)

Sharding: data-parallel over 8 NeuronCores, 32 batches (8192 tokens) per core.
Token-to-tile mapping: stat column j holds tokens {S*p + j : p in 0..127} so
the final [128, 64] result tile stores contiguously to DRAM.
The plain sums alternate DVE/ACT (5:3) to balance engine load; everything
is sized to hide under the ~70 us/core HBM read of x.
"""

import os

import numpy as np

import concourse.bacc as bacc
import concourse.bass as bass
import concourse.tile as tile
from concourse import mybir
from concourse.bass_utils import run_bass_kernel_spmd

B, N, E = 256, 257, 768
N_CORES = 8
BS = B // N_CORES          # batches per core
T = BS * (N - 1)           # tokens per core = 8192
P = 128                    # partitions
S = T // P                 # stat columns per core = 64
EPS = 1e-5
COLS_PER_DMA = 2

_CACHE = {}
LAST_RESULTS = None        # test harness reads exec_time_ns off this


def _build_nc(n_cols=S):
    S_ = n_cols
    T_ = P * S_
    J = COLS_PER_DMA
    nc = bacc.Bacc(None, target_bir_lowering=False)
    f32 = mybir.dt.float32

    x = nc.dram_tensor("x", [T_, E], f32, kind="ExternalInput")
    # params: [:, :768] = g2 replicated across partitions, [:, 768] = c
    params = nc.dram_tensor("params", [P, E + 1], f32, kind="ExternalInput")
    out = nc.dram_tensor("out", [T_], f32, kind="ExternalOutput")

    # x_r[s][p, j, e] = x[S_*p + J*s + j, e]
    x_r = x.ap().rearrange("(p s j) e -> s p j e", p=P, j=J)
    out_r = out.ap().rearrange("(p s) -> p s", p=P)

    with tile.TileContext(nc) as tc:
        with (
            tc.tile_pool(name="singles", bufs=1) as singles,
            tc.tile_pool(name="loads", bufs=4) as loads,
            tc.tile_pool(name="work", bufs=4) as work,
            tc.tile_pool(name="stats", bufs=1) as stats_pool,
        ):
            params_t = singles.tile([P, E + 1], f32)
            nc.sync.dma_start(out=params_t, in_=params.ap())
            g2_t = params_t[:, 0:E]
            c_ap = params_t[:, E : E + 1]

            eps_t = singles.tile([P, 1], f32)
            nc.vector.memset(eps_t, EPS)

            s2 = stats_pool.tile([P, S_], f32)   # dot(h, g2)
            sm = stats_pool.tile([P, S_], f32)   # sum(h)
            sq = stats_pool.tile([P, S_], f32)   # sum(h^2)

            for s in range(S_ // J):
                x_t = loads.tile([P, J, E], f32)
                nc.sync.dma_start(out=x_t, in_=x_r[s])

                for j in range(J):
                    col = J * s + j
                    xj = x_t[:, j, :]

                    d_sq = work.tile([P, 1], f32, tag="d_sq")
                    nc.scalar.activation(
                        out=d_sq.broadcast_to(xj.shape),
                        in_=xj,
                        func=mybir.ActivationFunctionType.Square,
                        accum_out=sq[:, col : col + 1],
                    )

                    if col % 8 < 3:  # ACT takes 3 of 8 plain sums
                        d_sm = work.tile([P, 1], f32, tag="d_sm")
                        nc.scalar.activation(
                            out=d_sm.broadcast_to(xj.shape),
                            in_=xj,
                            func=mybir.ActivationFunctionType.Copy,
                            accum_out=sm[:, col : col + 1],
                        )
                    else:
                        d_sm = work.tile([P, 1], f32, tag="d_sm")
                        nc.vector.tensor_scalar(
                            out=d_sm.broadcast_to(xj.shape),
                            in0=xj,
                            scalar1=1.0,
                            scalar2=0.0,
                            op0=mybir.AluOpType.mult,
                            op1=mybir.AluOpType.add,
                            accum_out=sm[:, col : col + 1],
                        )

                    d_s2 = work.tile([P, 1], f32, tag="d_s2")
                    nc.vector.scalar_tensor_tensor(
                        out=d_s2.broadcast_to(xj.shape),
                        in0=xj,
                        scalar=1.0,
                        in1=g2_t,
                        op0=mybir.AluOpType.mult,
                        op1=mybir.AluOpType.mult,
                        accum_out=s2[:, col : col + 1],
                    )

            # epilogue, batched over all tokens of the core: [128, S_]
            mu = stats_pool.tile([P, S_], f32)
            nc.vector.tensor_scalar(
                out=mu, in0=sm, scalar1=1.0 / E, scalar2=None,
                op0=mybir.AluOpType.mult,
            )
            musq = stats_pool.tile([P, S_], f32)
            nc.vector.tensor_mul(out=musq, in0=mu, in1=mu)
            var = stats_pool.tile([P, S_], f32)
            nc.vector.scalar_tensor_tensor(
                out=var, in0=sq, scalar=1.0 / E, in1=musq,
                op0=mybir.AluOpType.mult, op1=mybir.AluOpType.subtract,
            )
            std = stats_pool.tile([P, S_], f32)
            nc.scalar.activation(
                out=std, in_=var,
                func=mybir.ActivationFunctionType.Sqrt,
                bias=eps_t, scale=1.0,
            )
            r = stats_pool.tile([P, S_], f32)
            nc.vector.reciprocal(out=r, in_=std)
            logit = stats_pool.tile([P, S_], f32)
            nc.vector.tensor_mul(out=logit, in0=s2, in1=r)
            res = stats_pool.tile([P, S_], f32)
            nc.scalar.activation(
                out=res, in_=logit,
                func=mybir.ActivationFunctionType.Sigmoid,
                bias=c_ap, scale=1.0,
            )
            nc.sync.dma_start(out=out_r, in_=res)

    nc.compile()
    return nc


def kernel(x, ln_gamma, ln_beta, W, bias):
    global LAST_RESULTS
    x = np.ascontiguousarray(np.asarray(x, dtype=np.float32))
    ln_gamma = np.asarray(ln_gamma, dtype=np.float32)
    ln_beta = np.asarray(ln_beta, dtype=np.float32)
    W = np.asarray(W, dtype=np.float32)
    bias = np.asarray(bias, dtype=np.float32)

    geff = ln_gamma * W[0]
    g2 = geff - geff.sum() / E
    c = float(ln_beta @ W[0] + bias[0])

    params = np.empty((P, E + 1), dtype=np.float32)
    params[:, :E] = g2[None, :]
    params[:, E] = c

    # drop CLS, shard over cores, flatten to [T, E] per core
    h = x[:, 1:, :]                                  # [256, 256, 768]
    shards = [
        np.ascontiguousarray(h[i * BS : (i + 1) * BS].reshape(T, E))
        for i in range(N_CORES)
    ]

    if "nc" not in _CACHE:
        _CACHE["nc"] = _build_nc()
    nc = _CACHE["nc"]

    in_maps = [{"x": shards[i], "params": params} for i in range(N_CORES)]
    trace = bool(int(os.environ.get("BASS_KERNEL_TRACE", "0")))
    results = run_bass_kernel_spmd(
        nc, in_maps, core_ids=list(range(N_CORES)), trace=trace
    )
    LAST_RESULTS = results

    outs = [results.results[i]["out"] for i in range(N_CORES)]
    full = np.concatenate(outs).reshape(B, N - 1, 1).astype(np.float32)
    return full


# revision 11
# speedup vs baseline: 1.3662x; 1.3256x over previous
"""Trainium2 Bass kernel for a ClassificationHead:
  h = x[:, 1:, :]                      # drop CLS token
  h = LayerNorm(h) * gamma + beta      # over last dim (768)
  logits = h @ W.T + bias              # W: [1, 768]
  out = sigmoid(logits)                # [256, 256, 1]

Math reformulation (everything becomes per-token reductions over e=768):
  geff = gamma * W[0]
  g2   = geff - sum(geff)/768    # folds the LN mean-correction into the weights
  c    = dot(beta, W[0]) + bias[0]
  s2[t]  = dot(h[t], g2)
  var[t] = population variance of h[t]
  out[t] = sigmoid(s2[t] / sqrt(var[t] + eps) + c)

Sharding: data-parallel over 8 NeuronCores, 32 batches (8192 tokens) per core.
Token-to-column mapping: stat column `col` holds tokens {64*p + col} so the
final [128, 64] result tile stores contiguously to DRAM.

Engine split (balanced so each engine hides under the ~70us/core HBM read):
  - DVE: the g2-dot for every column (scalar_tensor_tensor accum), plus
    bn_stats/bn_aggr (mean+var in one pass) for 3 of every 8 columns, plus
    a few plain sums for fine balance.
  - ACT: Square-accum (sum of squares) + Copy-accum (plain sum) for the
    remaining 5 of 8 columns; Sqrt/Sigmoid epilogue.
  - Columns are interleaved bn/ACT at period 8 so both engines stream
    concurrently; ACT tables are pre-warmed to keep the epilogue short.
"""

import os

import numpy as np

import concourse.bacc as bacc
import concourse.bass as bass
import concourse.tile as tile
from concourse import mybir
from concourse.bass_utils import run_bass_kernel_spmd

B, N, E = 256, 257, 768
N_CORES = 8
BS = B // N_CORES          # batches per core
T = BS * (N - 1)           # tokens per core = 8192
P = 128                    # partitions
S = T // P                 # stat columns per core = 64
EPS = 1e-5

_CACHE = {}
LAST_RESULTS = None        # test harness reads exec_time_ns off this


def _build_nc():
    nc = bacc.Bacc(None, target_bir_lowering=False)
    f32 = mybir.dt.float32
    J = 2                       # columns per DMA
    G = 8                       # column group size for the bn/ACT pattern
    K = 3                       # bn columns per group
    NG = S // G
    n_act = G - K

    x = nc.dram_tensor("x", [T, E], f32, kind="ExternalInput")
    # params: [:, :768] = g2 replicated across partitions, [:, 768] = c
    params = nc.dram_tensor("params", [P, E + 1], f32, kind="ExternalInput")
    out = nc.dram_tensor("out", [T], f32, kind="ExternalOutput")
    # x_rj[s][p, :] = rows {S*p + J*s + j} of x, contiguous per partition
    x_rj = x.ap().rearrange("(p s j) e -> s p (j e)", p=P, j=J)
    out_r = out.ap().rearrange("(p s) -> p s", p=P)

    with tile.TileContext(nc) as tc:
        with (
            tc.tile_pool(name="singles", bufs=1) as singles,
            tc.tile_pool(name="loads", bufs=6) as loads,
            tc.tile_pool(name="work", bufs=3) as work,
            tc.tile_pool(name="stats", bufs=1) as stats_pool,
        ):
            params_t = singles.tile([P, E + 1], f32)
            nc.sync.dma_start(out=params_t, in_=params.ap())
            g2_t = params_t[:, 0:E]
            c_ap = params_t[:, E : E + 1]
            eps_t = singles.tile([P, 1], f32)
            nc.vector.memset(eps_t, EPS)

            # pre-warm the Sqrt/Sigmoid ACT tables so the epilogue doesn't
            # pay two serial ~1.3us lazy table loads
            warm = singles.tile([P, 1], f32)
            nc.scalar.activation(
                out=warm, in_=eps_t,
                func=mybir.ActivationFunctionType.Sqrt, bias=eps_t, scale=1.0,
            )
            nc.scalar.activation(
                out=warm, in_=warm,
                func=mybir.ActivationFunctionType.Sigmoid, bias=0.0, scale=1.0,
            )

            s2 = stats_pool.tile([P, S], f32)
            mv = stats_pool.tile([P, NG, K, 2], f32, name="mv")
            sm = stats_pool.tile([P, NG, n_act], f32, name="smt")
            sq = stats_pool.tile([P, NG, n_act], f32, name="sqt")

            for s in range(S // J):
                x_t = loads.tile([P, J * E], f32)
                nc.sync.dma_start(out=x_t, in_=x_rj[s])

                for j in range(J):
                    col = J * s + j
                    g, i = col // G, col % G
                    xj = x_t[:, j * E : (j + 1) * E]

                    if i < K:
                        # mean+var in one DVE pass (two 384-wide bn_stats)
                        x2 = xj.rearrange("p (h f) -> p h f", h=2)
                        st = work.tile([P, 2, 6], f32, tag="bnstats")
                        for h in range(2):
                            nc.vector.bn_stats(out=st[:, h, :], in_=x2[:, h, :])
                        nc.vector.bn_aggr(out=mv[:, g, i, :], in_=st)
                    else:
                        ac = i - K
                        d_sq = work.tile([P, 1], f32, tag="d_sq")
                        nc.scalar.activation(
                            out=d_sq.broadcast_to(xj.shape), in_=xj,
                            func=mybir.ActivationFunctionType.Square,
                            accum_out=sq[:, g, ac : ac + 1],
                        )
                        if col % 16 == 7:   # fine balance: DVE takes 4 sums
                            d_sm = work.tile([P, 1], f32, tag="d_sm")
                            nc.vector.tensor_scalar(
                                out=d_sm.broadcast_to(xj.shape), in0=xj,
                                scalar1=1.0, scalar2=0.0,
                                op0=mybir.AluOpType.mult,
                                op1=mybir.AluOpType.add,
                                accum_out=sm[:, g, ac : ac + 1],
                            )
                        else:
                            d_sm = work.tile([P, 1], f32, tag="d_sm")
                            nc.scalar.activation(
                                out=d_sm.broadcast_to(xj.shape), in_=xj,
                                func=mybir.ActivationFunctionType.Copy,
                                accum_out=sm[:, g, ac : ac + 1],
                            )

                    d = work.tile([P, 1], f32, tag="d")
                    nc.vector.scalar_tensor_tensor(
                        out=d.broadcast_to(xj.shape), in0=xj, scalar=1.0,
                        in1=g2_t,
                        op0=mybir.AluOpType.mult, op1=mybir.AluOpType.mult,
                        accum_out=s2[:, col : col + 1],
                    )

            # epilogue: assemble var in column order [P, NG, G]
            var = stats_pool.tile([P, NG, G], f32, name="var")
            nc.vector.tensor_copy(var[:, :, 0:K], mv[:, :, :, 1])
            mu = stats_pool.tile([P, NG, n_act], f32, name="mu")
            nc.vector.tensor_scalar(
                out=mu, in0=sm, scalar1=1.0 / E, scalar2=None,
                op0=mybir.AluOpType.mult,
            )
            musq = stats_pool.tile([P, NG, n_act], f32, name="musq")
            nc.vector.tensor_mul(out=musq, in0=mu, in1=mu)
            nc.vector.scalar_tensor_tensor(
                out=var[:, :, K:G], in0=sq, scalar=1.0 / E, in1=musq,
                op0=mybir.AluOpType.mult, op1=mybir.AluOpType.subtract,
            )
            varf = var.rearrange("p a b -> p (a b)")
            std = stats_pool.tile([P, S], f32, name="std")
            nc.scalar.activation(
                out=std, in_=varf, func=mybir.ActivationFunctionType.Sqrt,
                bias=eps_t, scale=1.0,
            )
            r = stats_pool.tile([P, S], f32, name="r")
            nc.vector.reciprocal(out=r, in_=std)
            logit = stats_pool.tile([P, S], f32, name="logit")
            nc.vector.tensor_mul(out=logit, in0=s2, in1=r)
            res = stats_pool.tile([P, S], f32, name="res")
            nc.scalar.activation(
                out=res, in_=logit, func=mybir.ActivationFunctionType.Sigmoid,
                bias=c_ap, scale=1.0,
            )
            nc.sync.dma_start(out=out_r, in_=res)

    nc.compile()
    return nc


def kernel(x, ln_gamma, ln_beta, W, bias):
    global LAST_RESULTS
    x = np.ascontiguousarray(np.asarray(x, dtype=np.float32))
    ln_gamma = np.asarray(ln_gamma, dtype=np.float32)
    ln_beta = np.asarray(ln_beta, dtype=np.float32)
    W = np.asarray(W, dtype=np.float32)
    bias = np.asarray(bias, dtype=np.float32)

    geff = ln_gamma * W[0]
    g2 = geff - geff.sum() / E
    c = float(ln_beta @ W[0] + bias[0])

    params = np.empty((P, E + 1), dtype=np.float32)
    params[:, :E] = g2[None, :]
    params[:, E] = c

    # drop CLS, shard over cores, flatten to [T, E] per core
    h = x[:, 1:, :]                                  # [256, 256, 768]
    shards = [
        np.ascontiguousarray(h[i * BS : (i + 1) * BS].reshape(T, E))
        for i in range(N_CORES)
    ]

    if "nc" not in _CACHE:
        _CACHE["nc"] = _build_nc()
    nc = _CACHE["nc"]

    in_maps = [{"x": shards[i], "params": params} for i in range(N_CORES)]
    trace = bool(int(os.environ.get("BASS_KERNEL_TRACE", "0")))
    results = run_bass_kernel_spmd(
        nc, in_maps, core_ids=list(range(N_CORES)), trace=trace
    )
    LAST_RESULTS = results

    outs = [results.results[i]["out"] for i in range(N_CORES)]
    full = np.concatenate(outs).reshape(B, N - 1, 1).astype(np.float32)
    return full


# revision 12
# speedup vs baseline: 1.3762x; 1.0073x over previous
"""Trainium2 Bass kernel for a ClassificationHead:
  h = x[:, 1:, :]                      # drop CLS token
  h = LayerNorm(h) * gamma + beta      # over last dim (768)
  logits = h @ W.T + bias              # W: [1, 768]
  out = sigmoid(logits)                # [256, 256, 1]

Math reformulation (everything becomes per-token reductions over e=768):
  geff = gamma * W[0]
  g2   = geff - sum(geff)/768    # folds the LN mean-correction into the weights
  c    = dot(beta, W[0]) + bias[0]
  s2[t]  = dot(h[t], g2)
  var[t] = population variance of h[t]
  out[t] = sigmoid(s2[t] / sqrt(var[t] + eps) + c)

Sharding: data-parallel over 8 NeuronCores, 32 batches (8192 tokens) per core.
Token-to-column mapping: stat column `col` holds tokens {64*p + col} so the
final [128, 64] result tile stores contiguously to DRAM.

Engine split (balanced so each engine hides under the ~70us/core HBM read):
  - DVE: the g2-dot for every column (scalar_tensor_tensor accum), plus
    bn_stats/bn_aggr (mean+var in one pass) for 3 of every 8 columns, plus
    a few plain sums for fine balance.
  - ACT: Square-accum (sum of squares) + Copy-accum (plain sum) for the
    remaining 5 of 8 columns; Sqrt/Sigmoid epilogue.
  - Columns are interleaved bn/ACT at period 8 so both engines stream
    concurrently; ACT tables are pre-warmed to keep the epilogue short.
"""

import os

import numpy as np

import concourse.bacc as bacc
import concourse.bass as bass
import concourse.tile as tile
from concourse import mybir
from concourse.bass_utils import run_bass_kernel_spmd

B, N, E = 256, 257, 768
N_CORES = 8
BS = B // N_CORES          # batches per core
T = BS * (N - 1)           # tokens per core = 8192
P = 128                    # partitions
S = T // P                 # stat columns per core = 64
EPS = 1e-5

_CACHE = {}
LAST_RESULTS = None        # test harness reads exec_time_ns off this


def _build_nc():
    nc = bacc.Bacc(None, target_bir_lowering=False)
    f32 = mybir.dt.float32
    J = 2                       # columns per DMA
    G = 8                       # column group size for the bn/ACT pattern
    K = 3                       # bn columns per group
    NG = S // G
    n_act = G - K

    x = nc.dram_tensor("x", [T, E], f32, kind="ExternalInput")
    # params: [:, :768] = g2 replicated across partitions, [:, 768] = c
    params = nc.dram_tensor("params", [P, E + 1], f32, kind="ExternalInput")
    out = nc.dram_tensor("out", [T], f32, kind="ExternalOutput")
    # x_rj[s][p, :] = rows {S*p + J*s + j} of x, contiguous per partition
    x_rj = x.ap().rearrange("(p s j) e -> s p (j e)", p=P, j=J)
    out_r = out.ap().rearrange("(p s) -> p s", p=P)

    with tile.TileContext(nc) as tc:
        with (
            tc.tile_pool(name="singles", bufs=1) as singles,
            tc.tile_pool(name="loads", bufs=8) as loads,
            tc.tile_pool(name="work", bufs=3) as work,
            tc.tile_pool(name="stats", bufs=1) as stats_pool,
        ):
            params_t = singles.tile([P, E + 1], f32)
            g2_t = params_t[:, 0:E]
            c_ap = params_t[:, E : E + 1]
            eps_t = singles.tile([P, 1], f32)
            nc.vector.memset(eps_t, EPS)

            # pre-warm the Sqrt/Sigmoid ACT tables so the epilogue doesn't
            # pay two serial ~1.3us lazy table loads
            warm = singles.tile([P, 1], f32)
            nc.scalar.activation(
                out=warm, in_=eps_t,
                func=mybir.ActivationFunctionType.Sqrt, bias=eps_t, scale=1.0,
            )
            nc.scalar.activation(
                out=warm, in_=warm,
                func=mybir.ActivationFunctionType.Sigmoid, bias=0.0, scale=1.0,
            )

            s2 = stats_pool.tile([P, S], f32)
            mv = stats_pool.tile([P, NG, K, 2], f32, name="mv")
            sm = stats_pool.tile([P, NG, n_act], f32, name="smt")
            sq = stats_pool.tile([P, NG, n_act], f32, name="sqt")

            for s in range(S // J):
                x_t = loads.tile([P, J * E], f32)
                nc.sync.dma_start(out=x_t, in_=x_rj[s])
                if s == 0:
                    # params gate only the dots (not bn_stats); loading them
                    # second lets compute start one transfer earlier
                    nc.sync.dma_start(out=params_t, in_=params.ap())

                for j in range(J):
                    col = J * s + j
                    g, i = col // G, col % G
                    xj = x_t[:, j * E : (j + 1) * E]

                    if i < K:
                        # mean+var in one DVE pass (two 384-wide bn_stats)
                        x2 = xj.rearrange("p (h f) -> p h f", h=2)
                        st = work.tile([P, 2, 6], f32, tag="bnstats")
                        for h in range(2):
                            nc.vector.bn_stats(out=st[:, h, :], in_=x2[:, h, :])
                        nc.vector.bn_aggr(out=mv[:, g, i, :], in_=st)
                    else:
                        ac = i - K
                        d_sq = work.tile([P, 1], f32, tag="d_sq")
                        nc.scalar.activation(
                            out=d_sq.broadcast_to(xj.shape), in_=xj,
                            func=mybir.ActivationFunctionType.Square,
                            accum_out=sq[:, g, ac : ac + 1],
                        )
                        if col % 16 == 7:   # fine balance: DVE takes 4 sums
                            d_sm = work.tile([P, 1], f32, tag="d_sm")
                            nc.vector.tensor_scalar(
                                out=d_sm.broadcast_to(xj.shape), in0=xj,
                                scalar1=1.0, scalar2=0.0,
                                op0=mybir.AluOpType.mult,
                                op1=mybir.AluOpType.add,
                                accum_out=sm[:, g, ac : ac + 1],
                            )
                        else:
                            d_sm = work.tile([P, 1], f32, tag="d_sm")
                            nc.scalar.activation(
                                out=d_sm.broadcast_to(xj.shape), in_=xj,
                                func=mybir.ActivationFunctionType.Copy,
                                accum_out=sm[:, g, ac : ac + 1],
                            )

                    d = work.tile([P, 1], f32, tag="d")
                    nc.vector.scalar_tensor_tensor(
                        out=d.broadcast_to(xj.shape), in0=xj, scalar=1.0,
                        in1=g2_t,
                        op0=mybir.AluOpType.mult, op1=mybir.AluOpType.mult,
                        accum_out=s2[:, col : col + 1],
                    )

            # epilogue: assemble var in column order [P, NG, G]
            var = stats_pool.tile([P, NG, G], f32, name="var")
            nc.vector.tensor_copy(var[:, :, 0:K], mv[:, :, :, 1])
            mu = stats_pool.tile([P, NG, n_act], f32, name="mu")
            nc.vector.tensor_scalar(
                out=mu, in0=sm, scalar1=1.0 / E, scalar2=None,
                op0=mybir.AluOpType.mult,
            )
            musq = stats_pool.tile([P, NG, n_act], f32, name="musq")
            nc.vector.tensor_mul(out=musq, in0=mu, in1=mu)
            nc.vector.scalar_tensor_tensor(
                out=var[:, :, K:G], in0=sq, scalar=1.0 / E, in1=musq,
                op0=mybir.AluOpType.mult, op1=mybir.AluOpType.subtract,
            )
            varf = var.rearrange("p a b -> p (a b)")
            std = stats_pool.tile([P, S], f32, name="std")
            nc.scalar.activation(
                out=std, in_=varf, func=mybir.ActivationFunctionType.Sqrt,
                bias=eps_t, scale=1.0,
            )
            r = stats_pool.tile([P, S], f32, name="r")
            nc.vector.reciprocal(out=r, in_=std)
            logit = stats_pool.tile([P, S], f32, name="logit")
            nc.vector.tensor_mul(out=logit, in0=s2, in1=r)
            res = stats_pool.tile([P, S], f32, name="res")
            nc.scalar.activation(
                out=res, in_=logit, func=mybir.ActivationFunctionType.Sigmoid,
                bias=c_ap, scale=1.0,
            )
            nc.sync.dma_start(out=out_r, in_=res)

    nc.compile()
    return nc


def kernel(x, ln_gamma, ln_beta, W, bias):
    global LAST_RESULTS
    x = np.ascontiguousarray(np.asarray(x, dtype=np.float32))
    ln_gamma = np.asarray(ln_gamma, dtype=np.float32)
    ln_beta = np.asarray(ln_beta, dtype=np.float32)
    W = np.asarray(W, dtype=np.float32)
    bias = np.asarray(bias, dtype=np.float32)

    geff = ln_gamma * W[0]
    g2 = geff - geff.sum() / E
    c = float(ln_beta @ W[0] + bias[0])

    params = np.empty((P, E + 1), dtype=np.float32)
    params[:, :E] = g2[None, :]
    params[:, E] = c

    # drop CLS, shard over cores, flatten to [T, E] per core
    h = x[:, 1:, :]                                  # [256, 256, 768]
    shards = [
        np.ascontiguousarray(h[i * BS : (i + 1) * BS].reshape(T, E))
        for i in range(N_CORES)
    ]

    if "nc" not in _CACHE:
        _CACHE["nc"] = _build_nc()
    nc = _CACHE["nc"]

    in_maps = [{"x": shards[i], "params": params} for i in range(N_CORES)]
    trace = bool(int(os.environ.get("BASS_KERNEL_TRACE", "0")))
    results = run_bass_kernel_spmd(
        nc, in_maps, core_ids=list(range(N_CORES)), trace=trace
    )
    LAST_RESULTS = results

    outs = [results.results[i]["out"] for i in range(N_CORES)]
    full = np.concatenate(outs).reshape(B, N - 1, 1).astype(np.float32)
    return full
